# revision 25
# baseline (speedup 1.0000x reference)
"""Cross-attention kernel for 8 TRN2 NeuronCores.

Problem: B=4, T_V=8192, T_T=77, C=1024, H=16, D=64 (f32).
  q = video @ Wq.T ; k,v = text @ W.T ; out = softmax(qk/sqrt(D)) v @ Wo.T

Sharding: data-parallel over (batch, T_V/2) -> 8 shards of [4096, 1024].
Each core gets its video shard, its batch's text, and all weights.
No collectives.

On-chip dataflow (everything "transposed": rows of video on the FREE dim):
  host pre-transposes and bf16-casts X -> X^T [C, M] and weights W^T
  [C, C] so the contraction dim always lands on SBUF partitions and
  DMA traffic halves. All GEMMs run bf16 x bf16 -> f32 PSUM at the
  full 1 row/cycle PE rate.
  Q^T = WqT-chunks . X^T ; K^T [C, T] and V [T, C] from text.
  Heads processed in pairs sharing one [128,512] PSUM tile:
    S^T_h = K_h^T . Q_h^T  -> exp on ScalarE (scale=1/8 folded in; no
    max-subtraction: scores are O(1) bounded)
    denominators via two 1-row PE matmuls against a ones vector into a
    [2,512] PSUM tile (the GPSIMD partition_all_reduce alternative
    measured ~2.7us/op on HW - way off its cost model - so the PE,
    which has slack, does the partition reduction instead)
    AV pair -> pso [128,512]; DVE copy frees PSUM early
    den copy (ScalarE) -> DVE approx-reciprocal (offset-0 APs only:
    the custom DVE uop mishandles nonzero partition offsets) -> one
    SBUF->SBUF broadcast DMA [2,512]->[128,512] -> one DVE multiply
    normalizes the pair straight into ot (bf16).
  out = O^T-chunks . WoT in natural [m, n] layout (PSUM -> SBUF on
  ScalarE, then DMA).
  Software pipelining: per iteration emit [xt-dma_{j+1}, Q_j, out_{j-1},
  attn_j] so attention post-ops drain while PE runs the next dense
  GEMMs, and out-proj never waits on normalization.
"""

import sys

if "/opt/trn_rl_repo" not in sys.path:
    sys.path.insert(0, "/opt/trn_rl_repo")

import numpy as np

import concourse.bacc as bacc
import concourse.bass as bass
import concourse.mybir as mybir
import concourse.tile as tile
from concourse.bass_utils import run_bass_kernel_spmd

F32 = mybir.dt.float32
F32R = mybir.dt.float32r
BF16 = mybir.dt.bfloat16
AF = mybir.ActivationFunctionType
ALU = mybir.AluOpType

B, T_V, T_T, C, H = 4, 8192, 77, 1024, 16
D = C // H            # 64
P = 128
KC = C // P           # 8 contraction chunks
M = T_V // 2          # 4096 rows per core
MB = 512              # m-block (rows processed per pipeline stage)
NBLK = M // MB        # 8
MSUB = MB // P        # 4 output row-chunks per block
T = T_T               # 77
TP = 80               # padded T for even-moving-dim matmuls
SCALE = 1.0 / float(np.sqrt(D))

_CACHED_NC = None

# scheduling knobs (swept in sim)
CFG = {
    "osb_copy": True,    # copy pso->SBUF before mult (frees PSUM early)
    "den_eng": "act",    # engine for den_ps -> den_sb copies
    "ob_eng": "dve",     # engine for out-proj PSUM->SBUF copies
    "defer_mult": 1,     # pairs to defer the normalize-mult by
    "lookahead": 0,      # S-matmul pairs emitted ahead of consumers
    "den_psmm": False,   # allocate den_ps from the shared psmm pool
    "pss_bufs": 3,
    "pso_bufs": 2,
}


def _build(repeat: int = 1):
    nc = bacc.Bacc(name="cross_attention")

    xt = nc.dram_tensor("xt", [C, M], BF16, kind="ExternalInput")
    yt = nc.dram_tensor("yt", [C, T], BF16, kind="ExternalInput")
    wqt = nc.dram_tensor("wqt", [C, C], BF16, kind="ExternalInput")
    wkt = nc.dram_tensor("wkt", [C, C], BF16, kind="ExternalInput")
    wvt = nc.dram_tensor("wvt", [C, C], BF16, kind="ExternalInput")
    wot = nc.dram_tensor("wot", [C, C], BF16, kind="ExternalInput")
    out = nc.dram_tensor("out", [M, C], F32, kind="ExternalOutput")

    # [C, X] dram views chunked to [P, KC, X]
    xt_v = xt[:, :].rearrange("(kc p) m -> p kc m", p=P)
    yt_v = yt[:, :].rearrange("(kc p) t -> p kc t", p=P)
    wq_v = wqt[:, :].rearrange("(kc p) n -> p kc n", p=P)
    wk_v = wkt[:, :].rearrange("(kc p) n -> p kc n", p=P)
    wv_v = wvt[:, :].rearrange("(kc p) n -> p kc n", p=P)
    wo_v = wot[:, :].rearrange("(kc p) n -> p kc n", p=P)

    from contextlib import ExitStack

    with tile.TileContext(nc) as tc, ExitStack() as st:
        pool = lambda *a, **k: st.enter_context(tc.tile_pool(*a, **k))
        if True:
            wq_pool = pool(name="wq", bufs=1)
            wo_pool = pool(name="wo", bufs=1)
            kt_pool = pool(name="kt", bufs=1)
            v_pool = pool(name="vv", bufs=1)
            wkv_pool = pool(name="wkv", bufs=1)
            yt_pool = pool(name="yt", bufs=1)
            xt_pool = pool(name="xt", bufs=2)
            qt_pool = pool(name="qt", bufs=2)
            ot_pool = pool(name="ot", bufs=2)
            es_pool = pool(name="es", bufs=4)
            osb_pool = pool(name="osb", bufs=3)
            dsb_pool = pool(name="dsb", bufs=3)
            rr2_pool = pool(name="rr2", bufs=3)
            rbb_pool = pool(name="rbb", bufs=3)
            ob_pool = pool(name="ob", bufs=3)
            ps_mm = pool(name="psmm", bufs=2, space="PSUM")
            ps_s = pool(name="pss", bufs=CFG["pss_bufs"], space="PSUM")
            ps_o = pool(name="pso", bufs=CFG["pso_bufs"], space="PSUM")
            ps_d = None if CFG["den_psmm"] else pool(name="psd", bufs=1, space="PSUM")

            wq_sb = wq_pool.tile([P, KC, C], BF16)
            wo_sb = wo_pool.tile([P, KC, C], BF16)
            kt_sb = kt_pool.tile([P, KC, T], BF16)
            v_sb = v_pool.tile([T, H, D], BF16)
            ones_sb = v_pool.tile([T, 2], BF16, tag="ones")
            nc.vector.memset(ones_sb[:], 1.0)

            xt_tiles = {}

            def emit_xt_dma(j):
                xt_t = xt_pool.tile([P, KC, MB], BF16, tag="xt")
                xt_tiles[j] = xt_t
                nc.sync.dma_start(xt_t[:], xt_v[:, :, j * MB : (j + 1) * MB])

            qt_tiles = {}

            def emit_qproj(j):
                qt_t = qt_pool.tile([P, KC, MB], BF16, tag="qt")
                qt_tiles[j] = qt_t
                xt_t = xt_tiles.pop(j)
                for nc_ in range(KC):
                    psq = ps_mm.tile([P, MB], F32, tag="mm")
                    for kc in range(KC):
                        nc.tensor.matmul(
                            psq[:],
                            wq_sb[:, kc, nc_ * P : (nc_ + 1) * P],
                            xt_t[:, kc, :],
                            start=(kc == 0),
                            stop=(kc == KC - 1),
                        )
                    nc.scalar.copy(out=qt_t[:, nc_, :], in_=psq[:])

            ot_tiles = {}

            def emit_attn(j):
                qt_t = qt_tiles.pop(j)
                ot_t = ot_pool.tile([P, KC, MB], BF16, tag="ot")
                ot_tiles[j] = ot_t
                pending = []
                sdone = []

                def do_mult():
                    dst, src_, rb_ = pending.pop(0)
                    nc.vector.tensor_tensor(dst, src_[:], rb_[:], ALU.mult)

                def emit_s_pair(jc):
                    ess = []
                    for hf in range(2):
                        pss = ps_s.tile([T, MB], F32, tag="pss")
                        nc.tensor.matmul(
                            pss[:],
                            kt_sb[64 * hf : 64 * hf + 64, jc, :],
                            qt_t[64 * hf : 64 * hf + 64, jc, :],
                            start=True,
                            stop=True,
                        )
                        es = es_pool.tile([T, MB], BF16, tag="es")
                        nc.scalar.activation(es[:], pss[:], AF.Exp, scale=SCALE)
                        ess.append(es)
                    sdone.append((jc, ess))

                def emit_rest():
                    jc, ess = sdone.pop(0)
                    pso = ps_o.tile([P, MB], F32, tag="pso")
                    # matmul outputs must start at partition 0/32/64: den of
                    # head pair lands on rows 0 and 64 of one PSUM bank
                    if CFG["den_psmm"]:
                        den_ps = ps_mm.tile([P, MB], F32, tag="mm", name="den_ps")
                    else:
                        den_ps = ps_d.tile([P, MB], F32, tag="den")
                    for hf in range(2):
                        # denominator: 1-row matmul against ones (PE does
                        # the partition reduction; GPSIMD allreduce is too
                        # slow on HW)
                        nc.tensor.matmul(
                            den_ps[64 * hf : 64 * hf + 1, :],
                            ones_sb[:, hf : hf + 1],
                            ess[hf][:],
                            start=True,
                            stop=True,
                        )
                    for hf in range(2):
                        nc.tensor.matmul(
                            pso[64 * hf : 64 * hf + 64, :],
                            v_sb[:, 2 * jc + hf, :],
                            ess[hf][:],
                            start=True,
                            stop=True,
                        )
                    # engine writes must start at partition 0/32/64, so the
                    # two denominators live on ONE partition as free columns
                    den_sb = dsb_pool.tile([1, 2, MB], F32, tag="dsb")
                    dcp = nc.vector.tensor_copy if CFG["den_eng"] == "dve" else (
                        lambda out, in_: nc.scalar.copy(out=out, in_=in_))
                    dcp(out=den_sb[:, 0, :], in_=den_ps[0:1, :])
                    dcp(out=den_sb[:, 1, :], in_=den_ps[64:65, :])
                    rr2 = rr2_pool.tile([1, 2, MB], F32, tag="rr2")
                    nc.vector.reciprocal_approx_fast(rr2[:], den_sb[:])
                    rb = rbb_pool.tile([P, MB], F32, tag="rbb")
                    for hf in range(2):
                        nc.sync.dma_start(
                            rb[64 * hf : 64 * hf + 64, :],
                            rr2[:, hf, None, :].to_broadcast((1, D, MB)),
                        )
                    if CFG["osb_copy"]:
                        o_sb = osb_pool.tile([P, MB], F32, tag="osb")
                        nc.vector.tensor_copy(out=o_sb[:], in_=pso[:])
                        pending.append((ot_t[:, jc, :], o_sb, rb))
                    else:
                        pending.append((ot_t[:, jc, :], pso, rb))
                    # defer the normalize-multiply so the broadcast DMA
                    # latency hides behind the next pair's DVE work
                    if len(pending) > CFG["defer_mult"]:
                        do_mult()

                for jc in range(KC):
                    emit_s_pair(jc)
                    if len(sdone) > CFG["lookahead"]:
                        emit_rest()
                while sdone:
                    emit_rest()
                while pending:
                    do_mult()

            def emit_outproj(j):
                ot_t = ot_tiles.pop(j)
                for mi in range(MSUB):
                    for nh in range(2):
                        pst = ps_mm.tile([P, MB], F32, tag="mm")
                        for cc in range(KC):
                            nc.tensor.matmul(
                                pst[:],
                                ot_t[:, cc, mi * P : (mi + 1) * P],
                                wo_sb[:, cc, nh * MB : (nh + 1) * MB],
                                start=(cc == 0),
                                stop=(cc == KC - 1),
                            )
                        ob = ob_pool.tile([P, MB], F32, tag="ob")
                        if CFG["ob_eng"] == "act":
                            nc.scalar.copy(out=ob[:], in_=pst[:])
                        else:
                            nc.vector.tensor_copy(out=ob[:], in_=pst[:])
                        nc.sync.dma_start(
                            out[
                                j * MB + mi * P : j * MB + (mi + 1) * P,
                                nh * MB : (nh + 1) * MB,
                            ],
                            ob[:],
                        )

            # ---- prologue: overlap weight DMAs with first-block compute ----
            blocks = [jj for _ in range(repeat) for jj in range(NBLK)]

            emit_xt_dma(blocks[0])
            yt_sb = yt_pool.tile([P, KC, TP], BF16)
            nc.vector.memset(yt_sb[:], 0.0)
            nc.sync.dma_start(yt_sb[:, :, :T], yt_v[:])
            for kc in range(KC):
                nc.sync.dma_start(wq_sb[:, kc, :], wq_v[:, kc, :])

            emit_qproj(blocks[0])

            wk_sb = wkv_pool.tile([P, KC, C], BF16, tag="wkv")
            for kc in range(KC):
                nc.sync.dma_start(wk_sb[:, kc, :], wk_v[:, kc, :])
            # K^T [C, T]: chunk nc_ holds rows 128*nc_..128*nc_+128
            for nc_ in range(KC):
                psk_full = ps_mm.tile([P, MB], F32, tag="mm", name="psk")
                psk = psk_full[:, :TP]
                for kc in range(KC):
                    nc.tensor.matmul(
                        psk[:],
                        wk_sb[:, kc, nc_ * P : (nc_ + 1) * P],
                        yt_sb[:, kc, :],
                        start=(kc == 0),
                        stop=(kc == KC - 1),
                    )
                nc.vector.tensor_copy(out=kt_sb[:, nc_, :], in_=psk[:, :T])

            if len(blocks) > 1:
                emit_xt_dma(blocks[1])

            wv_sb = wkv_pool.tile([P, KC, C], BF16, tag="wkv")
            for kc in range(KC):
                nc.sync.dma_start(wv_sb[:, kc, :], wv_v[:, kc, :])
            # V natural [T, C] written per 512-wide column slab into
            # the strided per-head layout v_sb[t, h, 0:64]
            for half in range(2):
                psv_full = ps_mm.tile([P, MB], F32, tag="mm", name="psv")
                psv = psv_full[:T, :]
                for kc in range(KC):
                    nc.tensor.matmul(
                        psv[:],
                        yt_sb[:, kc, :T],
                        wv_sb[:, kc, half * MB : (half + 1) * MB],
                        start=(kc == 0),
                        stop=(kc == KC - 1),
                    )
                nc.vector.tensor_copy(
                    out=v_sb[:, half * 8 : (half + 1) * 8, :],
                    in_=psv[:].rearrange("t (h d) -> t h d", d=D),
                )

            for kc in range(KC):
                nc.sync.dma_start(wo_sb[:, kc, :], wo_v[:, kc, :])

            # ---- software-pipelined main loop ----
            emit_attn(blocks[0])
            for i in range(1, len(blocks)):
                if i + 1 < len(blocks):
                    emit_xt_dma(blocks[i + 1])
                emit_qproj(blocks[i])
                emit_outproj(blocks[i - 1])
                emit_attn(blocks[i])
            emit_outproj(blocks[-1])
    nc.finalize()
    return nc


def _get_nc(repeat: int = 1):
    global _CACHED_NC
    if _CACHED_NC is None:
        _CACHED_NC = {}
    if repeat not in _CACHED_NC:
        _CACHED_NC[repeat] = _build(repeat)
    return _CACHED_NC[repeat]


def kernel(video_features, text_features, Wq, Wk, Wv, Wo, **_unused):
    import ml_dtypes

    bf16 = ml_dtypes.bfloat16
    video_features = np.asarray(video_features, dtype=np.float32)
    text_features = np.asarray(text_features, dtype=np.float32)
    wqt = np.ascontiguousarray(np.asarray(Wq, dtype=np.float32).T).astype(bf16)
    wkt = np.ascontiguousarray(np.asarray(Wk, dtype=np.float32).T).astype(bf16)
    wvt = np.ascontiguousarray(np.asarray(Wv, dtype=np.float32).T).astype(bf16)
    wot = np.ascontiguousarray(np.asarray(Wo, dtype=np.float32).T).astype(bf16)

    in_maps = []
    for c in range(8):
        b, half = divmod(c, 2)
        xs = video_features[b, half * M : (half + 1) * M, :]  # [M, C]
        in_maps.append(
            {
                "xt": np.ascontiguousarray(xs.T).astype(bf16),   # [C, M]
                "yt": np.ascontiguousarray(text_features[b].T).astype(bf16),
                "wqt": wqt,
                "wkt": wkt,
                "wvt": wvt,
                "wot": wot,
            }
        )

    res = run_bass_kernel_spmd(_get_nc(), in_maps, core_ids=list(range(8)))
    outf = np.empty((B, T_V, C), dtype=np.float32)
    for c in range(8):
        b, half = divmod(c, 2)
        outf[b, half * M : (half + 1) * M, :] = res.results[c]["out"]
    return outf
